# revision 7
# baseline (speedup 1.0000x reference)
"""MultiHeadDenseAttention on 8 Trainium2 NeuronCores.

Head-sharded tensor parallelism: each core computes 2 of 16 heads
(value projection slice, per-head MLP attention logits, softmax, S@V),
then an AllToAll exchanges head-blocks for row-blocks so each core
computes the output projection for its 512 rows with the full Wo.

v4: all-bf16 datapath (fp32 PSUM accumulation), PSUM-accumulated value
projection, stacked two-head hid matmul, bf16 AllToAll payloads,
persistent tile pools so consecutive reps pipeline (rep k's exchange +
output projection overlaps rep k+1's value/logits), and an s-streamed
output projection that starts as soon as the first source block is
normalized.

Layouts (per core c, heads 2c / 2c+1):
  xt   [1024, 4096] bf16  x.reshape(4096,1024).T  (feat on partitions)
  xc   [128, 4096]  bf16  xt rows [128c, 128c+128)
  wv   [128, 1024]  bf16  Wv[128c:+128,:].T chunked  lhsT for value proj
  w1t  [128, 128]   bf16  blockdiag(W1.T, W1.T)  stacked two-head hid
  w2t  [65, 2048]   bf16  W2.T with b2 as row 64
  hidT [65, 4096]   bf16  per head; row 64 = ones (pairs with b2 row)
  vh[b] [128, 16*130] bf16 transposed value chunks + ones cols
  logits psum [128m, 1024n] per m-chunk pair; exp -> bf16 tiles
  S@V: po[65, 512] = vh_aug.T @ expT  (row 64 = softmax denominator)
  A2A [8, 65, 512] bf16 per head; normalize + out proj after exchange.
"""

import sys

if "/opt/trn_rl_repo" not in sys.path:
    sys.path.insert(0, "/opt/trn_rl_repo")

from contextlib import ExitStack

import numpy as np

import bass_rust
import concourse.bass as bass
import concourse.tile as tile
from concourse import masks, mybir
from concourse.bass_utils import run_bass_kernel_spmd

F32 = mybir.dt.float32
BF16 = mybir.dt.bfloat16
AF = mybir.ActivationFunctionType

NC = 8            # cores
B = 2             # batch
N_SEQ = 2048      # seq len == max_seq_len (m)
FEAT = 1024
H = 16            # heads
D = 64            # head dim
NTOT = B * N_SEQ  # 4096 flattened rows
NBLK = 512        # n-block size
NB = NTOT // NBLK # 8 n-blocks (== A2A shards == cores)
MC = N_SEQ // 128 # 16 m-chunks per batch
CB = 130          # vh per-chunk stride: 65 (h0+ones) + 65 (h1+ones)


def _split_sem_waits(nc, limit=1):
    """Walrus rejects instructions with more than ~1 sync wait; move the
    excess onto NOPs on the same engine inserted immediately before."""
    blocks = {}
    for f in nc.m.functions:
        for bb in f.blocks:
            blocks[bb.name] = bb
    for bb in blocks.values():
        i = 0
        while i < len(bb.instructions):
            inst = bb.instructions[i]
            si = inst.sync_info
            if si is not None and si.on_wait and len(si.on_wait) > limit:
                waits = list(si.on_wait)
                chunks = [waits[j : j + limit] for j in range(0, len(waits), limit)]
                si.on_wait = chunks[-1]
                engine = nc.engines[inst.engine]
                for chunk in chunks[:-1]:
                    d = engine.nop(nofuse=True, hint="wait_split")
                    dinst = d.ins if hasattr(d, "ins") else d
                    for ob in blocks.values():
                        if ob.instructions and ob.instructions[-1] is dinst:
                            ob.instructions.pop()
                            break
                    dinst.sync_info = bass_rust.SyncInfo(on_wait=chunk, on_update=[])
                    bb.instructions.insert(i, dinst)
                    i += 1
            i += 1
    return nc


def _build(reps=1, phases="A"):
    nc = bass.Bass()

    xt_in = nc.dram_tensor("xt", [FEAT, NTOT], BF16, kind="ExternalInput")
    xc_in = nc.dram_tensor("xc", [128, NTOT], BF16, kind="ExternalInput")
    wv_in = nc.dram_tensor("wv", [128, FEAT], BF16, kind="ExternalInput")
    w1t_in = nc.dram_tensor("w1t", [128, 128], BF16, kind="ExternalInput")
    b1_in = nc.dram_tensor("b1", [128, 1], F32, kind="ExternalInput")
    w2t_in = nc.dram_tensor("w2t", [65, N_SEQ], BF16, kind="ExternalInput")
    wot_in = nc.dram_tensor("wot", [128, NC * FEAT], BF16, kind="ExternalInput")
    sel_in = nc.dram_tensor("sel", [2, 128], BF16, kind="ExternalInput")
    ones_in = nc.dram_tensor("onesr", [1, NTOT], BF16, kind="ExternalInput")
    out_ext = nc.dram_tensor("out", [NBLK, FEAT], F32, kind="ExternalOutput")

    with tile.TileContext(nc) as tc, ExitStack() as ctx:
        wp = ctx.enter_context(tc.tile_pool(name="wp", bufs=1))
        dram = ctx.enter_context(tc.tile_pool(name="dram", bufs=1, space="DRAM"))

        # ---- resident weights/constants -------------------------------
        wv = wp.tile([128, FEAT], BF16)
        nc.sync.dma_start(wv[:], wv_in[:])
        w1t = wp.tile([128, 128], BF16)
        nc.sync.dma_start(w1t[:], w1t_in[:])
        b1t = wp.tile([128, 1], F32)
        nc.sync.dma_start(b1t[:], b1_in[:])
        w2t = wp.tile([65, N_SEQ], BF16)
        nc.sync.dma_start(w2t[:], w2t_in[:])
        xc = wp.tile([128, NTOT], BF16)
        nc.sync.dma_start(xc[:], xc_in[:])
        sel = wp.tile([2, 128], BF16)
        nc.sync.dma_start(sel[:], sel_in[:])
        wot = wp.tile([128, NC * FEAT], BF16)
        nc.sync.dma_start(wot[:], wot_in[:])

        ident_f = wp.tile([128, 128], F32)
        masks.make_identity(nc, ident_f[:])
        ident = wp.tile([128, 128], BF16)
        nc.vector.tensor_copy(ident[:], ident_f[:])
        onecol_f = wp.tile([128, 1], F32)
        nc.vector.memset(onecol_f[:], 1.0)

        vh = [wp.tile([128, MC * CB], BF16, name=f"vh{b}", tag=f"vh{b}") for b in range(B)]
        # constant ones columns (softmax-denominator trick), written once
        for b in range(B):
            for j in range(MC):
                nc.vector.tensor_copy(vh[b][:, j * CB + D : j * CB + D + 1], onecol_f[:])
                nc.vector.tensor_copy(vh[b][:, j * CB + 65 + D : j * CB + 65 + D + 1], onecol_f[:])

        # ---- persistent pools (cross-rep pipelining) ------------------
        psm = ctx.enter_context(tc.tile_pool(name="psm", bufs=2, space="PSUM"))
        psl = ctx.enter_context(tc.tile_pool(name="psl", bufs=2, space="PSUM"))
        pso = ctx.enter_context(tc.tile_pool(name="pso", bufs=1, space="PSUM"))
        psw = ctx.enter_context(tc.tile_pool(name="psw", bufs=1, space="PSUM"))
        hp = ctx.enter_context(tc.tile_pool(name="hp", bufs=2))
        ep = ctx.enter_context(tc.tile_pool(name="ep", bufs=4))
        op = ctx.enter_context(tc.tile_pool(name="op", bufs=4))
        vap = ctx.enter_context(tc.tile_pool(name="vap", bufs=1))
        xfp = ctx.enter_context(tc.tile_pool(name="xfp", bufs=3))
        rp = ctx.enter_context(tc.tile_pool(name="rp", bufs=6))
        awp = ctx.enter_context(tc.tile_pool(name="awp", bufs=1))
        obp = ctx.enter_context(tc.tile_pool(name="obp", bufs=2))

        def emit_tail(a2a_recv):
            # ---- P4: normalize per source; P5: s-streamed out proj ----
            actw = []
            for s in range(NC):
                sums = rp.tile([2, NBLK], F32, tag="sums", name="sums")
                nc.gpsimd.dma_start(sums[0:1, :], a2a_recv[0][s, D : D + 1, :])
                nc.gpsimd.dma_start(sums[1:2, :], a2a_recv[1][s, D : D + 1, :])
                raw = rp.tile([128, NBLK], BF16, tag="raw", name="raw")
                nc.gpsimd.dma_start(raw[0:D, :], a2a_recv[0][s, 0:D, :])
                nc.gpsimd.dma_start(raw[D:128, :], a2a_recv[1][s, 0:D, :])
                rcps_f = rp.tile([2, NBLK], F32, tag="rcpf", name="rcpf")
                nc.vector.reciprocal(rcps_f[:], sums[:])
                rcps = rp.tile([2, NBLK], BF16, tag="rcp", name="rcp")
                nc.vector.tensor_copy(rcps[:], rcps_f[:])
                pb = psm.tile([128, NBLK], F32, tag="pm", name="pb")
                nc.tensor.matmul(
                    pb[:], sel[:], rcps[:], start=True, stop=True,
                    skip_group_check=True,
                )
                aw = awp.tile([128, NBLK], BF16, tag=f"aw{s}", name=f"aw{s}")
                actw.append(aw)
                nc.vector.tensor_mul(aw[:], raw[:], pb[:])

            for t in range(NBLK // 128):
                ob = obp.tile([128, FEAT], F32, tag="ob", name="ob")
                for half in range(2):
                    pw = psw.tile([128, NBLK], F32, tag="pw", name=f"pw{t}_{half}")
                    for s in range(NC):
                        nc.tensor.matmul(
                            pw[:],
                            actw[s][:, t * 128 : (t + 1) * 128],
                            wot[:, s * FEAT + half * NBLK : s * FEAT + (half + 1) * NBLK],
                            start=(s == 0),
                            stop=(s == NC - 1),
                            skip_group_check=True,
                        )
                    nc.vector.tensor_copy(ob[:, half * NBLK : (half + 1) * NBLK], pw[:])
                nc.gpsimd.dma_start(out_ext[t * 128 : (t + 1) * 128, :], ob[:])

        pending_tail = None
        for _rep in range(reps):
            a2a_send = [dram.tile([NC, 65, NBLK], BF16, name=f"snd{h}_{_rep}") for h in range(2)]
            a2a_recv = [dram.tile([NC, 65, NBLK], BF16, name=f"rcv{h}_{_rep}") for h in range(2)]

            # ---- hid MLP: both heads stacked on 128 partitions --------
            hidTs = []
            for h in range(2):
                hidT = hp.tile([65, NTOT], BF16, name=f"hidT{h}", tag="hidT")
                hidTs.append(hidT)
                nc.sync.dma_start(hidT[D : D + 1, :], ones_in[:])
            for nb in range(NB):
                ph = psm.tile([128, NBLK], F32, tag="pm", name="ph")
                nc.tensor.matmul(
                    ph[:],
                    w1t[:],
                    xc[:, nb * NBLK : (nb + 1) * NBLK],
                    start=True,
                    stop=True,
                    skip_group_check=True,
                )
                for h in range(2):
                    nc.scalar.activation(
                        hidTs[h][0:D, nb * NBLK : (nb + 1) * NBLK],
                        ph[h * D : (h + 1) * D, :],
                        AF.Relu,
                        bias=b1t[h * D : (h + 1) * D, :],
                    )

            # ---- P1: value projection, PSUM accumulation --------------
            vacc = vap.tile([128, NTOT], BF16, tag="vacc")
            for nb in range(NB):
                xs = xfp.tile([128, 8 * NBLK], BF16, tag="xs", name="xs")
                for f in range(8):
                    nc.sync.dma_start(
                        xs[:, f * NBLK : (f + 1) * NBLK],
                        xt_in[f * 128 : (f + 1) * 128, nb * NBLK : (nb + 1) * NBLK],
                    )
                pv = psm.tile([128, NBLK], F32, tag="pm", name="pv")
                for f in range(8):
                    nc.tensor.matmul(
                        pv[:],
                        wv[:, f * 128 : (f + 1) * 128],
                        xs[:, f * NBLK : (f + 1) * NBLK],
                        start=(f == 0),
                        stop=(f == 7),
                        skip_group_check=True,
                    )
                dst = vacc[:, nb * NBLK : (nb + 1) * NBLK]
                nc.vector.tensor_copy(dst, pv[:])
                # previous rep's exchange has landed by now — drain its
                # normalize + output projection here so the PE never stalls
                # on the collective inside its in-order queue
                if nb == 5 and pending_tail is not None:
                    emit_tail(pending_tail)
                    pending_tail = None
                # transpose this block's m-chunks into vh
                b = nb // (NB // B)
                for ji in range(4):
                    j = (nb % 4) * 4 + ji
                    tp = psm.tile([128, 128], BF16, tag="pm", name=f"tp{nb}_{ji}")
                    nc.tensor.matmul(
                        tp[:],
                        vacc[:, b * N_SEQ + j * 128 : b * N_SEQ + (j + 1) * 128],
                        ident[:],
                        is_transpose=True,
                        start=True,
                        stop=True,
                    )
                    base = j * CB
                    nc.vector.tensor_copy(vh[b][:, base : base + D], tp[:, 0:D])
                    nc.vector.tensor_copy(vh[b][:, base + 65 : base + 65 + D], tp[:, D:128])

            # ---- P2: attention ----------------------------------------
            for h in range(2):
                hidT = hidTs[h]
                for nb in range(NB):
                    b = nb // (NB // B)
                    eqs = []
                    for qt in range(4):
                        eq = ep.tile([128, 4 * NBLK], BF16, name="expTq", tag="expTq")
                        eqs.append(eq)
                        for jj in range(0, 4, 2):
                            j = qt * 4 + jj
                            pl = psl.tile([128, 2 * NBLK], F32, tag="pl", name="pl")
                            for q in range(2):
                                nc.tensor.matmul(
                                    pl[:, q * NBLK : (q + 1) * NBLK],
                                    w2t[:, (j + q) * 128 : (j + q + 1) * 128],
                                    hidT[:, nb * NBLK : (nb + 1) * NBLK],
                                    start=True,
                                    stop=True,
                                    skip_group_check=True,
                                )
                            nc.scalar.activation(
                                eq[:, jj * NBLK : (jj + 2) * NBLK], pl[:], AF.Exp
                            )
                    po = pso.tile([65, NBLK], F32, tag="po", name="po")
                    for j in range(MC):
                        nc.tensor.matmul(
                            po[:],
                            vh[b][:, j * CB + h * 65 : j * CB + (h + 1) * 65],
                            eqs[j // 4][:, (j % 4) * NBLK : (j % 4 + 1) * NBLK],
                            start=(j == 0),
                            stop=(j == MC - 1),
                            skip_group_check=True,
                        )
                    ot = op.tile([65, NBLK], BF16, tag="ot", name="ot")
                    nc.vector.tensor_copy(ot[:], po[:])
                    nc.sync.dma_start(a2a_send[h][nb], ot[:])

                # fire this head's exchange as soon as its blocks are out
                if phases not in ("1", "2"):
                    nc.gpsimd.collective_compute(
                        "AllToAll",
                        mybir.AluOpType.bypass,
                        ins=[a2a_send[h][:].opt()],
                        outs=[a2a_recv[h][:].opt()],
                        replica_groups=[list(range(NC))],
                    )

            if phases in ("1", "2", "3"):
                continue
            pending_tail = a2a_recv

        if pending_tail is not None:
            emit_tail(pending_tail)

    _split_sem_waits(nc)
    return nc


_CACHE = {}


def _get_program(reps=1, phases="A"):
    key = ("nc", reps, phases)
    if key not in _CACHE:
        _CACHE[key] = _build(reps, phases)
    return _CACHE[key]


def _bf16(x):
    import jax.numpy as jnp

    return np.asarray(jnp.asarray(np.asarray(x, np.float32)).astype(jnp.bfloat16))


def kernel(x, W1, b1, W2, b2, Wv, Wo, _run_kwargs=None):
    x = np.asarray(x, dtype=np.float32)
    W1 = np.asarray(W1, dtype=np.float32)
    b1 = np.asarray(b1, dtype=np.float32)
    W2 = np.asarray(W2, dtype=np.float32)
    b2 = np.asarray(b2, dtype=np.float32)
    Wv = np.asarray(Wv, dtype=np.float32)
    Wo = np.asarray(Wo, dtype=np.float32)

    xt = _bf16(x.reshape(NTOT, FEAT).T)                       # [1024, 4096]
    w1blk = np.zeros((128, 128), dtype=np.float32)            # blockdiag(W1.T, W1.T)
    w1blk[0:D, 0:D] = W1.T
    w1blk[D:128, D:128] = W1.T
    w1t = _bf16(w1blk)
    w2t = _bf16(np.concatenate([W2.T, b2.reshape(1, N_SEQ)], axis=0))  # [65, 2048]
    wot = _bf16(
        Wo.T.reshape(NC, 128, FEAT).transpose(1, 0, 2).reshape(128, NC * FEAT)
    )
    b1s = np.ascontiguousarray(
        np.concatenate([b1, b1]).reshape(128, 1), dtype=np.float32
    )
    sel_h = np.zeros((2, 128), dtype=np.float32)
    sel_h[0, :D] = 1.0
    sel_h[1, D:] = 1.0
    sel_h = _bf16(sel_h)
    onesr = _bf16(np.ones((1, NTOT), dtype=np.float32))

    in_maps = []
    for c in range(NC):
        wv_c = _bf16(
            Wv[c * 128 : (c + 1) * 128, :].T
            .reshape(8, 128, 128).transpose(1, 0, 2).reshape(128, FEAT)
        )
        in_maps.append(
            {
                "xt": xt,
                "xc": np.ascontiguousarray(xt[c * 128 : (c + 1) * 128, :]),
                "wv": wv_c,
                "w1t": w1t,
                "b1": b1s,
                "w2t": w2t,
                "wot": wot,
                "sel": sel_h,
                "onesr": onesr,
            }
        )

    import os
    nc = _get_program(
        int(os.environ.get("KERNEL_REPS", "1")), os.environ.get("KERNEL_PHASES", "A")
    )
    res = run_bass_kernel_spmd(
        nc, in_maps, list(range(NC)), **(_run_kwargs or {})
    )
    out = np.concatenate([res.results[c]["out"] for c in range(NC)], axis=0)
    if _run_kwargs:
        kernel.last_results = res
    return out.reshape(B, N_SEQ, FEAT)
